# revision 17
# baseline (speedup 1.0000x reference)
"""MoE (MiniMax decoder MLP) Trainium2 kernel — expert-parallel across 8 NeuronCores.

Strategy (per the expert-parallel sharding hint):
  - Host computes the router (softmax + top-2 + renormalize) — this IS the
    sharding decision — and dispatches each token's activation row to the
    core(s) owning its selected expert(s).
  - Core e holds expert e's weights [H,I]/[H,I]/[I,H] and computes
    silu(x @ Wg) * (x @ Wu) @ Wd for its routed tokens (padded to a common
    capacity C), scaling rows by the renormalized combine weight on-device.
  - Host scatter-adds the per-expert outputs back into the full [T, H] output.

Compute is done in bf16 on the TensorEngine (fp32 PSUM accumulation).

All device inputs are host-swizzled into SBUF-image layouts so every DMA is a
large per-partition-contiguous transfer (DMA issue slots, not bandwidth, bound
the start-up): gate/up weights are i-block-major and stream in interleaved
groups so the PE's first accumulation groups start ~9us into the kernel;
activations are window-major (one DMA per 512-token window) and issued from
the scalar queue (the second HWDGE engine) so they don't serialize behind the
weight DMAs on the sync queue. A few matmuls on garbage data warm the PE's
HAM clock gate during the DMA preamble.
"""

import os
import sys

import numpy as np

_EXTRA_PATHS = [
    "/root/.axon_site",
    "/root/.axon_site/_ro/trn_rl_repo",
    "/root/.axon_site/_ro/pypackages",
    "/opt/trn_rl_repo",
    "/opt/pypackages",
]
try:
    import concourse.bass  # noqa: F401
except ImportError:  # pragma: no cover
    sys.path[:0] = [p for p in _EXTRA_PATHS if p not in sys.path]

import ml_dtypes

B, S, H = 4, 2048, 2048
I = 1408  # expert intermediate size
E = 8  # num experts
K = 2  # experts per token
N_CORES = 8

KT = H // 128  # 16 contraction tiles over H
IT = I // 128  # 11 tiles over I
HW = H // 512  # 4 output windows over H
WBLK = KT * 128  # free-dim span of one i-block in the wg/wu SBUF image

_NC_CACHE = {}


def _windows(C):
    # Token windows <= 512 wide. Keep every window >= 256 wide (narrower
    # windows make matmul1 LDWEIGHTS-bound: ~88ns/LDW vs N*0.42ns/MM) while
    # keeping sum(ceil(W/128)) minimal (each m-tile costs a full matmul2
    # sweep regardless of how many tokens it holds).
    ws = []
    o = 0
    rem = C
    while rem > 0:
        if rem >= 512 + 256 or rem <= 512:
            w = min(512, rem)
        else:  # 513..767: split as [256, 257..511]
            w = 256
        ws.append((o, w))
        o += w
        rem -= w
    return ws


def _build_nc(C):
    """Build + compile the per-core expert MLP program for capacity C tokens."""
    import concourse.mybir as mybir
    import concourse.tile as tile
    from concourse import bacc

    fp32 = mybir.dt.float32
    bf16 = mybir.dt.bfloat16
    mult = mybir.AluOpType.mult
    silu_fn = mybir.ActivationFunctionType.Silu

    MT = -(-C // 128)  # token m-tiles (last may be partial)
    windows = _windows(C)

    nc = bacc.Bacc("TRN2", target_bir_lowering=False, debug=False, num_devices=N_CORES)

    # All inputs pre-swizzled to SBUF-image layouts (see kernel() below).
    xt = nc.dram_tensor("xt", [128, KT * C], bf16, kind="ExternalInput")
    wg = nc.dram_tensor("wg", [128, IT * WBLK], bf16, kind="ExternalInput")
    wu = nc.dram_tensor("wu", [128, IT * WBLK], bf16, kind="ExternalInput")
    wd = nc.dram_tensor("wd", [128, IT * H], bf16, kind="ExternalInput")
    cw = nc.dram_tensor("cw", [128, MT], fp32, kind="ExternalInput")
    out = nc.dram_tensor("out", [C, H], fp32, kind="ExternalOutput")

    with tile.TileContext(nc) as tc:
        with (
            tc.tile_pool(name="wpool", bufs=1) as wpool,
            tc.tile_pool(name="xpool", bufs=2) as xpool,
            tc.tile_pool(name="gpool", bufs=2) as gpool,
            tc.tile_pool(name="spool", bufs=2) as spool,
            tc.tile_pool(name="opool", bufs=3) as opool,
            tc.tile_pool(name="cwpool", bufs=1) as cwpool,
            tc.tile_pool(name="warm", bufs=1) as warm,
            tc.tile_pool(name="pgp", bufs=2, space="PSUM") as pgp,
            tc.tile_pool(name="pup", bufs=2, space="PSUM") as pup,
            tc.tile_pool(name="pop", bufs=3, space="PSUM") as pop,
            tc.tile_pool(name="pwp", bufs=1, space="PSUM") as pwp,
        ):
            # PE warm-up on garbage SBUF data: gets the HAM clock gate to
            # 8/8 while the first DMAs are still in flight. Never read back.
            wsrc = warm.tile([128, 512], bf16, name="wsrc", tag="wsrc")
            nc.vector.memset(wsrc[:], 1.0)
            pw = pwp.tile([128, 512], fp32, name="pw", tag="pw")
            for r in range(8):
                nc.tensor.matmul(
                    pw[:], wsrc[:, :128], wsrc[:], start=(r == 0), stop=(r == 7)
                )

            def dma_xt_window(o, W, nsplit, tail_engine=None):
                t = xpool.tile([128, KT * 512], bf16, name="xt_sb", tag="xt_sb")
                span = KT * W
                step = -(-KT // nsplit) * W  # chunk k-tiles per sub-DMA
                chunks = [(s0, min(s0 + step, span)) for s0 in range(0, span, step)]
                # The scalar queue issues serially (~0.6us each); handing the
                # last chunks to the idle gpsimd queue overlaps their issue.
                ntail = 2 if tail_engine is not None else 0
                for s0, s1 in chunks[: len(chunks) - ntail]:
                    nc.scalar.dma_start(
                        t[:, s0:s1], xt.ap()[:, KT * o + s0 : KT * o + s1]
                    )
                for s0, s1 in chunks[len(chunks) - ntail :]:
                    tail_engine.dma_start(
                        t[:, s0:s1], xt.ap()[:, KT * o + s0 : KT * o + s1]
                    )
                return t

            # Window-0 activations issue first on the scalar queue, split in
            # eight so the first gate matmuls start as early as possible.
            # (One DMA instruction lands on ONE of the 16 HW queues, so
            # bandwidth requires many concurrent mid-size transfers.)
            xt0_sb = dma_xt_window(*windows[0], nsplit=8, tail_engine=nc.gpsimd)

            # Combine weights [128, MT]: element [p, n] = weight of token
            # n*128+p. Issued after window-0's activations (cw is only needed
            # by the first down-proj scale, ~75us in).
            cw_sb = cwpool.tile([128, MT], fp32, name="cw_sb", tag="cw_sb")
            nc.scalar.dma_start(cw_sb[:], cw.ap()[:])

            # Expert weights (bf16) on the sync queue: interleaved gate/up
            # i-blocks (0.5MB each), then the down-proj image blocks.
            wg_sb = wpool.tile([128, IT * WBLK], bf16, name="wg_sb", tag="wg_sb")
            wu_sb = wpool.tile([128, IT * WBLK], bf16, name="wu_sb", tag="wu_sb")
            wd_sb = wpool.tile([128, IT * H], bf16, name="wd_sb", tag="wd_sb")
            for i in range(IT):
                nc.sync.dma_start(
                    wg_sb[:, i * WBLK : (i + 1) * WBLK],
                    wg.ap()[:, i * WBLK : (i + 1) * WBLK],
                )
                nc.sync.dma_start(
                    wu_sb[:, i * WBLK : (i + 1) * WBLK],
                    wu.ap()[:, i * WBLK : (i + 1) * WBLK],
                )
            for i in range(IT):
                nc.sync.dma_start(
                    wd_sb[:, i * H : (i + 1) * H], wd.ap()[:, i * H : (i + 1) * H]
                )

            def emit_matmul1(xt_sb, W):
                """silu(x@Wg) * (x@Wu) for one token window -> gated^T tiles."""
                gated = []
                for i in range(IT):
                    pg = pgp.tile([128, 512], fp32, name="pg", tag="pg")
                    pu = pup.tile([128, 512], fp32, name="pu", tag="pu")
                    for k in range(KT):
                        nc.tensor.matmul(
                            pg[:, :W],
                            wg_sb[:, i * WBLK + k * 128 : i * WBLK + (k + 1) * 128],
                            xt_sb[:, k * W : (k + 1) * W],
                            start=(k == 0),
                            stop=(k == KT - 1),
                        )
                    for k in range(KT):
                        nc.tensor.matmul(
                            pu[:, :W],
                            wu_sb[:, i * WBLK + k * 128 : i * WBLK + (k + 1) * 128],
                            xt_sb[:, k * W : (k + 1) * W],
                            start=(k == 0),
                            stop=(k == KT - 1),
                        )
                    act = spool.tile([128, 512], fp32, name="act", tag="act")
                    nc.scalar.activation(act[:, :W], pg[:, :W], silu_fn)
                    g = gpool.tile([128, 512], bf16, name=f"g{i}", tag=f"g{i}")
                    nc.vector.tensor_tensor(g[:, :W], act[:, :W], pu[:, :W], mult)
                    gated.append(g)
                return gated

            def emit_matmul2(o, W, gated, last=False):
                # Down-proj: out[tokens, H] accumulated over I, then scale by
                # the per-token combine weight. Out-DMAs are split across both
                # HWDGE issue engines and several queues: a single 256KB tile
                # on one queue (~22GB/s) would trail the last matmul by ~12us.
                nsp = 4 if last else 2
                for m in range(-(-W // 128)):
                    mg = o // 128 + m
                    P = min(128, W - m * 128)  # partial final m-tile
                    for h in range(HW):
                        po = pop.tile([128, 512], fp32, name="po", tag="po")
                        for i in range(IT):
                            nc.tensor.matmul(
                                po[:P, :],
                                gated[i][:, m * 128 : m * 128 + P],
                                wd_sb[:, i * H + h * 512 : i * H + (h + 1) * 512],
                                start=(i == 0),
                                stop=(i == IT - 1),
                            )
                        ob = opool.tile([128, 512], fp32, name="ob", tag="ob")
                        nc.vector.tensor_scalar_mul(
                            ob[:P, :], po[:P, :], cw_sb[:P, mg : mg + 1]
                        )
                        # Row-split (keeps 2KB DMA lines; the queues are
                        # descriptor-rate-bound, so column splits don't help).
                        rstep = -(-P // nsp)
                        for c, r0 in enumerate(range(0, P, rstep)):
                            r1 = min(r0 + rstep, P)
                            eng = nc.sync if c % 2 == 0 else nc.scalar
                            eng.dma_start(
                                out.ap()[
                                    o + m * 128 + r0 : o + m * 128 + r1,
                                    h * 512 : (h + 1) * 512,
                                ],
                                ob[r0:r1, :],
                            )

            # Window pipeline: matmul2 of window t is emitted after matmul1 of
            # window t+1 (gpool bufs=2 keeps both windows' gated tiles live),
            # so the start-up down-matmuls don't stall on the wd load.
            pending = None
            for wi, (o, W) in enumerate(windows):
                xt_sb = xt0_sb if wi == 0 else dma_xt_window(o, W, nsplit=4)
                gated = emit_matmul1(xt_sb, W)
                if pending is not None:
                    emit_matmul2(*pending)
                pending = (o, W, gated)
            emit_matmul2(*pending, last=True)

    nc.compile()
    return nc


def kernel(
    hidden_states: np.ndarray,
    gate_w: np.ndarray,
    w_gate: np.ndarray,
    w_up: np.ndarray,
    w_down: np.ndarray,
) -> np.ndarray:
    from concourse.bass_utils import run_bass_kernel_spmd

    x = np.asarray(hidden_states, dtype=np.float32).reshape(-1, H)
    gate_w = np.asarray(gate_w, dtype=np.float32)
    w_gate = np.asarray(w_gate, dtype=np.float32)
    w_up = np.asarray(w_up, dtype=np.float32)
    w_down = np.asarray(w_down, dtype=np.float32)
    T = x.shape[0]

    # Router (the sharding decision): softmax over experts, top-2, renormalize.
    logits = x @ gate_w.T
    logits -= logits.max(axis=-1, keepdims=True)
    ex = np.exp(logits)
    probs = ex / ex.sum(axis=-1, keepdims=True)
    topk_i = np.argpartition(-probs, K - 1, axis=-1)[:, :K]  # [T, K]
    topk_w = np.take_along_axis(probs, topk_i, axis=-1)
    denom = topk_w.sum(axis=-1)  # [T]

    sels, cws = [], []
    for e in range(E):
        sel = np.nonzero((topk_i == e).any(axis=1))[0]
        sels.append(sel)
        cws.append(probs[sel, e] / denom[sel])

    max_count = max(len(s) for s in sels)
    C = max(128, max_count)
    MT = -(-C // 128)
    windows = _windows(C)

    if C not in _NC_CACHE:
        _NC_CACHE[C] = _build_nc(C)
    nc = _NC_CACHE[C]

    # Dispatch: gather each expert's tokens (transposed, bf16) + weights,
    # swizzled into the SBUF-image layouts the kernel's DMAs expect.
    xt_full = np.ascontiguousarray(x.T.astype(ml_dtypes.bfloat16))  # [H, T]

    def swz_w(w):  # [H, I] -> [128, IT*KT*128] i-block-major image
        return np.ascontiguousarray(
            w.astype(ml_dtypes.bfloat16)
            .reshape(KT, 128, IT, 128)
            .transpose(1, 2, 0, 3)
            .reshape(128, IT * KT * 128)
        )

    def swz_wd(w):  # [I, H] -> [128, IT*H] i-block-major image
        return np.ascontiguousarray(
            w.astype(ml_dtypes.bfloat16)
            .reshape(IT, 128, H)
            .transpose(1, 0, 2)
            .reshape(128, IT * H)
        )

    def swz_xt(xpad):  # [H, C] -> [128, KT*C] window-major image
        blocks = [
            xpad[:, o : o + W].reshape(KT, 128, W).transpose(1, 0, 2).reshape(128, -1)
            for o, W in windows
        ]
        return np.ascontiguousarray(np.concatenate(blocks, axis=1))

    in_maps = []
    for e in range(E):
        sel = sels[e]
        xpad = np.zeros((H, C), dtype=ml_dtypes.bfloat16)
        xpad[:, : len(sel)] = xt_full[:, sel]
        cw_e = np.zeros((128, MT), dtype=np.float32)
        cw_flat = np.zeros(MT * 128, dtype=np.float32)
        cw_flat[: len(sel)] = cws[e]
        cw_e[:] = cw_flat.reshape(MT, 128).T
        in_maps.append(
            {
                "xt": swz_xt(xpad),
                "wg": swz_w(w_gate[e]),
                "wu": swz_w(w_up[e]),
                "wd": swz_wd(w_down[e]),
                "cw": cw_e,
            }
        )

    trace = bool(os.environ.get("BASS_MOE_TRACE"))
    res = run_bass_kernel_spmd(
        nc, in_maps, core_ids=list(range(N_CORES)), trace=trace
    )
    if trace and res.exec_time_ns is not None:
        print(f"HW exec time: {res.exec_time_ns} ns")

    # Combine: scatter-add each expert's (already weight-scaled) rows.
    out_full = np.zeros((T, H), dtype=np.float32)
    for e in range(E):
        sel = sels[e]
        out_full[sel] += res.results[e]["out"][: len(sel)]
    return out_full.reshape(B, S, H)
